# revision 2
# baseline (speedup 1.0000x reference)
"""bf16 4-way PE-quadrant conv, whole-image SBUF residency, bf16 output.

Layout: the full image lives in one SBUF tile [128, 130, 258] bf16 per core.
Partitions 0-63 (lo) hold channels for image rows -1..128 (slot 0 dummy);
partitions 64-127 (up) hold rows 127..256 (slot 129 dummy). Columns padded by
one on each side. Border outputs consume dummy/pad garbage but are overlaid
from the separately-computed edge tensor on the host, as in v1.

Main conv (d=4): per 4-row group, 9 taps x 4 concurrent 64x64 bf16 quadrant
matmuls (lo/up half x PSUM col half). Output accumulates into two persistent
bf16 tiles DMA'd out in 4-group chunks. Edge pixels computed at the end
directly from the resident image (no separate edge_in upload).
"""

import ml_dtypes
import numpy as np

import concourse.bacc as bacc
import concourse.mybir as mybir
import concourse.tile as tile
from concourse.bass import ts
from concourse.bass_utils import run_bass_kernel_spmd

B, C, H, W = 8, 64, 256, 256
NCORES = 8
H2 = H // 2          # rows per partition-half
SLOTS = H2 + 2       # 130
WPAD = W + 2         # 258
XCOLS = SLOTS * WPAD
NG = H2 // 4         # 32 groups of 4 rows per half
OCOLS = NG * 512     # 16384 output cols per half
F32 = mybir.dt.float32
BF16 = mybir.dt.bfloat16
AF = mybir.ActivationFunctionType
BF = ml_dtypes.bfloat16

TAPS9 = [(dy, dx) for dy in (-1, 0, 1) for dx in (-1, 0, 1)]
TOP_TAPS = [(dy, dx) for dy in (0, 1) for dx in (-1, 0, 1)]      # d=7 row 0
BOT_TAPS = [(dy, dx) for dy in (-1, 0) for dx in (-1, 0, 1)]     # d=1 row 255
LEFT_TAPS = [(dy, dx) for dy in (-1, 0, 1) for dx in (0, 1)]     # d=5 col 0
RIGHT_TAPS = [(dy, dx) for dy in (-1, 0, 1) for dx in (-1, 0)]   # d=3 col 255
C6_TAPS = [(dy, dx) for dy in (0, 1) for dx in (-1, 0)]          # d=6 (0,255)
C2_TAPS = [(dy, dx) for dy in (-1, 0) for dx in (0, 1)]          # d=2 (255,0)
C8_TAPS = [(dy, dx) for dy in (0, 1) for dx in (0, 1)]           # d=8 (0,0)
C0_TAPS = [(dy, dx) for dy in (-1, 0) for dx in (-1, 0)]         # d=0 (255,255)

W_GROUPS = [
    (4, TAPS9), (7, TOP_TAPS), (1, BOT_TAPS), (5, LEFT_TAPS),
    (3, RIGHT_TAPS), (6, C6_TAPS), (2, C2_TAPS), (8, C8_TAPS), (0, C0_TAPS),
]
_offs = []
_acc = 0
for _d, _taps in W_GROUPS:
    _offs.append(_acc)
    _acc += len(_taps)
(MAIN_S, TOP_S, BOT_S, LEFT_S, RIGHT_S, C6_S, C2_S, C8_S, C0_S) = _offs
NW = _acc  # 49
NWM = 9    # main taps at the front of wt
# bias tile [128, NB]: column -> (value on partitions 0-63, on 64-127)
BIAS_PAIRS = [(4, 4), (5, 5), (3, 3), (7, 1), (8, 2), (6, 0)]
B_MAIN, B_LEFT, B_RIGHT, B_TOPBOT, B_C82, B_C60 = range(6)
NB = 6

EOUT = 516  # edges_out: [0:128] left, [128:256] right, [256:512] top|bottom,
            # [512:514] corner j01 (d8|d2), [514:516] corner j254/255 (d6|d0)

# input DMA chunk boundaries (slot index); first chunk small for fast start
XCHUNKS = [0, 18, 46, 74, 102, SLOTS]
# output DMA every OG groups
OG = 4

_CACHE = {}


def _chain(nc, psd, wtr, wslice, slot0, taps, rhs_fn):
    n = len(taps)
    for k, (dy, dx) in enumerate(taps):
        nc.tensor.matmul(psd, wtr[wslice, ts(slot0 + k, 64)], rhs_fn(dy, dx),
                         start=(k == 0), stop=(k == n - 1),
                         skip_group_check=True)


def _build():
    nc = bacc.Bacc("TRN2", target_bir_lowering=False, debug=False,
                   num_devices=NCORES)
    ip = nc.dram_tensor("img_prep", [128, XCOLS], BF16,
                        kind="ExternalInput").ap()
    wtm_d = nc.dram_tensor("wtm", [128, NWM * 64], BF16,
                           kind="ExternalInput").ap()
    wte_d = nc.dram_tensor("wte", [128, (NW - NWM) * 64], BF16,
                           kind="ExternalInput").ap()
    bias_d = nc.dram_tensor("bias", [128, NB], F32, kind="ExternalInput").ap()
    out_d = nc.dram_tensor("out", [2, 128, OCOLS], BF16,
                           kind="ExternalOutput").ap()
    edg_d = nc.dram_tensor("edges", [128, EOUT], F32,
                           kind="ExternalOutput").ap()

    lo, up = slice(0, 64), slice(64, 128)

    with tile.TileContext(nc) as tc:
        with (
            tc.tile_pool(name="const", bufs=1) as constp,
            tc.tile_pool(name="psmain", bufs=4, space="PSUM") as pp,
        ):
            wt = constp.tile([128, NW * 64], BF16)
            nc.sync.dma_start(wt[:, 0:NWM * 64], wtm_d[:])
            X = constp.tile([128, XCOLS], BF16)
            for ci in range(len(XCHUNKS) - 1):
                a, b = XCHUNKS[ci] * WPAD, XCHUNKS[ci + 1] * WPAD
                nc.sync.dma_start(X[:, a:b], ip[:, a:b])
            bias_t = constp.tile([128, NB], F32)
            nc.sync.dma_start(bias_t[:], bias_d[:])
            nc.sync.dma_start(wt[:, NWM * 64:], wte_d[:])
            wtr = wt[:]
            Xv = X[:].rearrange("p (t m) -> p t m", m=WPAD)

            olo = constp.tile([128, OCOLS], BF16)
            oup = constp.tile([128, OCOLS], BF16)

            # ---- dense interior conv (d=4) ----
            for g in range(NG):
                ps1 = pp.tile([128, 512], F32, tag="ps1")
                ps2 = pp.tile([128, 512], F32, tag="ps2")
                for k, (dy, dx) in enumerate(TAPS9):
                    st, sp = (k == 0), (k == 8)
                    for (ph, po, i) in ((lo, slice(0, 64), 4 * g),
                                        (up, slice(0, 64), 4 * g),
                                        (lo, slice(64, 128), 4 * g + 2),
                                        (up, slice(64, 128), 4 * g + 2)):
                        psd = (ps1 if ph == lo else ps2)
                        rhs = Xv[ph, i + 1 + dy: i + 3 + dy,
                                 dx + 1: dx + 257]
                        nc.tensor.matmul(
                            psd[po, :],
                            wtr[ph, ts(MAIN_S + k, 64)], rhs,
                            start=st, stop=sp, skip_group_check=True)
                # evacuate: bias add psum -> sbuf bf16; alternate engines
                blo = bias_t[:, B_MAIN:B_MAIN + 1]
                if g % 2 == 0:
                    nc.scalar.activation(olo[:, ts(g, 512)], ps1[:],
                                         AF.Identity, bias=blo)
                    nc.vector.tensor_scalar_add(oup[:, ts(g, 512)],
                                                ps2[:], blo)
                else:
                    nc.vector.tensor_scalar_add(olo[:, ts(g, 512)],
                                                ps1[:], blo)
                    nc.scalar.activation(oup[:, ts(g, 512)], ps2[:],
                                         AF.Identity, bias=blo)
                if g % OG == OG - 1:
                    a, b = (g - OG + 1) * 512, (g + 1) * 512
                    nc.sync.dma_start(out_d[0, :, a:b], olo[:, a:b])
                    nc.sync.dma_start(out_d[1, :, a:b], oup[:, a:b])

            # ---- edge computation (reads the resident image) ----
            esb = constp.tile([128, EOUT], F32)
            # left column (d=5): rows 0-127 on (lo,h0), rows 128-255 (up,h64)
            pside = pp.tile([128, 256], F32, tag="ps1")
            _chain(nc, pside[lo, 0:128], wtr, lo, LEFT_S, LEFT_TAPS,
                   lambda dy, dx: Xv[lo, 1 + dy: 129 + dy, dx + 1])
            _chain(nc, pside[up, 0:128], wtr, up, LEFT_S, LEFT_TAPS,
                   lambda dy, dx: Xv[up, 1 + dy: 129 + dy, dx + 1])
            nc.scalar.activation(esb[:, 0:128], pside[:, 0:128], AF.Identity,
                                 bias=bias_t[:, B_LEFT:B_LEFT + 1])
            # right column (d=3): img col 255+dx -> padded col 256+dx
            pside2 = pp.tile([128, 256], F32, tag="ps2")
            _chain(nc, pside2[lo, 0:128], wtr, lo, RIGHT_S, RIGHT_TAPS,
                   lambda dy, dx: Xv[lo, 1 + dy: 129 + dy, dx + 256])
            _chain(nc, pside2[up, 0:128], wtr, up, RIGHT_S, RIGHT_TAPS,
                   lambda dy, dx: Xv[up, 1 + dy: 129 + dy, dx + 256])
            nc.scalar.activation(esb[:, 128:256], pside2[:, 0:128],
                                 AF.Identity,
                                 bias=bias_t[:, B_RIGHT:B_RIGHT + 1])
            # top row (d=7) on lo / bottom row (d=1) on up
            ptb = pp.tile([128, 256], F32, tag="ps1")
            _chain(nc, ptb[lo, 0:256], wtr, lo, TOP_S, TOP_TAPS,
                   lambda dy, dx: Xv[lo, 1 + dy, dx + 1: dx + 257])
            _chain(nc, ptb[up, 0:256], wtr, up, BOT_S, BOT_TAPS,
                   lambda dy, dx: Xv[up, 128 + dy, dx + 1: dx + 257])
            nc.scalar.activation(esb[:, 256:512], ptb[:, 0:256], AF.Identity,
                                 bias=bias_t[:, B_TOPBOT:B_TOPBOT + 1])
            # corners: (0,0) d8 / (255,0) d2 at cols 512:514;
            #          (0,255) d6 / (255,255) d0 at cols 514:516
            pcn = pp.tile([128, 256], F32, tag="ps2")
            _chain(nc, pcn[lo, 0:2], wtr, lo, C8_S, C8_TAPS,
                   lambda dy, dx: Xv[lo, 1 + dy, dx + 1: dx + 3])
            _chain(nc, pcn[up, 0:2], wtr, up, C2_S, C2_TAPS,
                   lambda dy, dx: Xv[up, 128 + dy, dx + 1: dx + 3])
            _chain(nc, pcn[lo, 2:4], wtr, lo, C6_S, C6_TAPS,
                   lambda dy, dx: Xv[lo, 1 + dy, 255 + dx: 257 + dx])
            _chain(nc, pcn[up, 2:4], wtr, up, C0_S, C0_TAPS,
                   lambda dy, dx: Xv[up, 128 + dy, 255 + dx: 257 + dx])
            nc.scalar.activation(esb[:, 512:514], pcn[:, 0:2], AF.Identity,
                                 bias=bias_t[:, B_C82:B_C82 + 1])
            nc.scalar.activation(esb[:, 514:516], pcn[:, 2:4], AF.Identity,
                                 bias=bias_t[:, B_C60:B_C60 + 1])
            nc.sync.dma_start(edg_d[:], esb[:])

    nc.compile()
    return nc


def _get_nc():
    if "nc" not in _CACHE:
        _CACHE["nc"] = _build()
    return _CACHE["nc"]


def _prep_img(imgc):
    """[64,256,256] f32 -> [128, XCOLS] whole-image padded bf16 layout."""
    ipv = np.zeros((2, 64, SLOTS, WPAD), BF)
    ipv[0, :, 1:130, 1:257] = imgc[:, 0:129, :]     # lo: rows -1..128
    ipv[1, :, 0:129, 1:257] = imgc[:, 127:256, :]   # up: rows 127..256
    return np.ascontiguousarray(ipv.reshape(128, XCOLS))


def _prep_wt(weights):
    wt = np.zeros((128, NW, 64), BF)
    for (d, taps), base in zip(W_GROUPS, _offs):
        for k, (dy, dx) in enumerate(taps):
            m = weights[d][:, :, dy + 1, dx + 1].T  # [cin, cout]
            wt[0:64, base + k] = m
            wt[64:128, base + k] = m
    return np.ascontiguousarray(wt.reshape(128, NW * 64))


def _prep_bias(bias):
    bs = np.zeros((128, NB), np.float32)
    for c, (dl, du) in enumerate(BIAS_PAIRS):
        bs[0:64, c] = bias[dl]
        bs[64:128, c] = bias[du]
    return bs


def _make_in_maps(img, weights, bias):
    img = np.asarray(img, np.float32)
    wt = _prep_wt(np.asarray(weights, np.float32))
    wtm = np.ascontiguousarray(wt[:, :NWM * 64])
    wte = np.ascontiguousarray(wt[:, NWM * 64:])
    bs = _prep_bias(np.asarray(bias, np.float32))
    return [{"img_prep": _prep_img(img[c]), "wtm": wtm, "wte": wte,
             "bias": bs}
            for c in range(NCORES)]


def _unprep_out(o, e):
    """Assemble [C,H,W] f32 from dense bf16 out + f32 edge overlay."""
    v = o.reshape(2, 2, 64, NG, 2, 256)  # h pg c g r w
    out = np.ascontiguousarray(
        v.transpose(2, 0, 3, 1, 4, 5).reshape(C, H, W)).astype(np.float32)
    Lv = np.concatenate([e[0:64, 0:128], e[64:128, 0:128]], axis=1)
    Rv = np.concatenate([e[0:64, 128:256], e[64:128, 128:256]], axis=1)
    out[:, 1:255, 0] = Lv[:, 1:255]
    out[:, 1:255, 255] = Rv[:, 1:255]
    out[:, 0, 1:255] = e[0:64, 257:511]
    out[:, 255, 1:255] = e[64:128, 257:511]
    out[:, 0, 0] = e[0:64, 512]
    out[:, 255, 0] = e[64:128, 512]
    out[:, 0, 255] = e[0:64, 515]
    out[:, 255, 255] = e[64:128, 515]
    return out


def kernel(img, weights, bias):
    nc = _get_nc()
    in_maps = _make_in_maps(img, weights, bias)
    res = run_bass_kernel_spmd(nc, in_maps, list(range(NCORES)))
    return np.stack([_unprep_out(res.results[c]["out"],
                                 res.results[c]["edges"])
                     for c in range(NCORES)])


# revision 6
# speedup vs baseline: 1.1373x; 1.1373x over previous
"""bf16 4-way PE-quadrant conv, whole-image SBUF residency, bf16 output.

Layout: the full image lives in one SBUF tile [128, 130, 258] bf16 per core.
Partitions 0-63 (lo) hold channels for image rows -1..128 (slot 0 dummy);
partitions 64-127 (up) hold rows 127..256 (slot 129 dummy). Columns padded by
one on each side. Border outputs consume dummy/pad garbage but are overlaid
from the separately-computed edge tensor on the host.

Main conv (d=4): per 4-row group, 9 taps x 4 concurrent 64x64 bf16 quadrant
matmuls (lo/up half x PSUM col half). Output accumulates into two persistent
bf16 tiles DMA'd out every OG groups. Edge pixels are computed mid-loop
directly from the resident image (no separate edge_in upload).

DMA ordering: HW queues deliver packets in dispatch order, so small tensors
needed early (wtm, bias) are dispatched before the bulk image chunks.
"""

import ml_dtypes
import numpy as np

import concourse.bacc as bacc
import concourse.mybir as mybir
import concourse.tile as tile
from concourse.bass import ts
from concourse.bass_utils import run_bass_kernel_spmd

B, C, H, W = 8, 64, 256, 256
NCORES = 8
H2 = H // 2          # rows per partition-half
SLOTS = H2 + 2       # 130
WPAD = W + 2         # 258
XCOLS = SLOTS * WPAD
NG = H2 // 4         # 32 groups of 4 rows per half
OCOLS = NG * 512     # 16384 output cols per half
F32 = mybir.dt.float32
BF16 = mybir.dt.bfloat16
AF = mybir.ActivationFunctionType
BF = ml_dtypes.bfloat16

TAPS9 = [(dy, dx) for dy in (-1, 0, 1) for dx in (-1, 0, 1)]
TOP_TAPS = [(dy, dx) for dy in (0, 1) for dx in (-1, 0, 1)]      # d=7 row 0
BOT_TAPS = [(dy, dx) for dy in (-1, 0) for dx in (-1, 0, 1)]     # d=1 row 255
LEFT_TAPS = [(dy, dx) for dy in (-1, 0, 1) for dx in (0, 1)]     # d=5 col 0
RIGHT_TAPS = [(dy, dx) for dy in (-1, 0, 1) for dx in (-1, 0)]   # d=3 col 255
C6_TAPS = [(dy, dx) for dy in (0, 1) for dx in (-1, 0)]          # d=6 (0,255)
C2_TAPS = [(dy, dx) for dy in (-1, 0) for dx in (0, 1)]          # d=2 (255,0)
C8_TAPS = [(dy, dx) for dy in (0, 1) for dx in (0, 1)]           # d=8 (0,0)
C0_TAPS = [(dy, dx) for dy in (-1, 0) for dx in (-1, 0)]         # d=0 (255,255)

W_GROUPS = [
    (4, TAPS9), (7, TOP_TAPS), (1, BOT_TAPS), (5, LEFT_TAPS),
    (3, RIGHT_TAPS), (6, C6_TAPS), (2, C2_TAPS), (8, C8_TAPS), (0, C0_TAPS),
]
_offs = []
_acc = 0
for _d, _taps in W_GROUPS:
    _offs.append(_acc)
    _acc += len(_taps)
(MAIN_S, TOP_S, BOT_S, LEFT_S, RIGHT_S, C6_S, C2_S, C8_S, C0_S) = _offs
NW = _acc  # 49
NWM = 9    # main taps at the front of wt
# bias tile [128, NB]: column -> (value on partitions 0-63, on 64-127)
BIAS_PAIRS = [(4, 4), (5, 5), (3, 3), (7, 1), (8, 2), (6, 0)]
B_MAIN, B_LEFT, B_RIGHT, B_TOPBOT, B_C82, B_C60 = range(6)
NB = 6

EOUT = 516  # edges_out: [0:128] left, [128:256] right, [256:512] top|bottom,
            # [512:514] corner j01 (d8|d2), [514:516] corner j254/255 (d6|d0)

# input DMA chunk boundaries (slot index); first chunk small for fast start
XCHUNKS = [0, 10, 18, 46, 74, 102, SLOTS]
# output DMA every OG groups
OG = 2
# issue the edge computation after this many main-loop groups (image fully
# resident by then; keeps edge DMA latency off the kernel tail)
EDGE_AT = 18

_CACHE = {}


def _chain(nc, psd, wtr, wslice, slot0, taps, rhs_fn):
    n = len(taps)
    for k, (dy, dx) in enumerate(taps):
        nc.tensor.matmul(psd, wtr[wslice, ts(slot0 + k, 64)], rhs_fn(dy, dx),
                         start=(k == 0), stop=(k == n - 1),
                         skip_group_check=True)


def _edges(nc, pp, constp, wtr, bias_t, Xv, edg_d, lo, up):
    """Border-pixel conv chains, reading the resident image tile."""
    esb = constp.tile([128, EOUT], F32)
    # left column (d=5): rows 0-127 on (lo,h0), rows 128-255 on (up,h64)
    pside = pp.tile([128, 256], F32, tag="ps1")
    _chain(nc, pside[lo, 0:128], wtr, lo, LEFT_S, LEFT_TAPS,
           lambda dy, dx: Xv[lo, 1 + dy: 129 + dy, dx + 1])
    _chain(nc, pside[up, 0:128], wtr, up, LEFT_S, LEFT_TAPS,
           lambda dy, dx: Xv[up, 1 + dy: 129 + dy, dx + 1])
    nc.scalar.activation(esb[:, 0:128], pside[:, 0:128], AF.Identity,
                         bias=bias_t[:, B_LEFT:B_LEFT + 1])
    # right column (d=3): img col 255+dx -> padded col 256+dx
    pside2 = pp.tile([128, 256], F32, tag="ps2")
    _chain(nc, pside2[lo, 0:128], wtr, lo, RIGHT_S, RIGHT_TAPS,
           lambda dy, dx: Xv[lo, 1 + dy: 129 + dy, dx + 256])
    _chain(nc, pside2[up, 0:128], wtr, up, RIGHT_S, RIGHT_TAPS,
           lambda dy, dx: Xv[up, 1 + dy: 129 + dy, dx + 256])
    nc.scalar.activation(esb[:, 128:256], pside2[:, 0:128], AF.Identity,
                         bias=bias_t[:, B_RIGHT:B_RIGHT + 1])
    # top row (d=7) on lo / bottom row (d=1) on up
    ptb = pp.tile([128, 256], F32, tag="ps1")
    _chain(nc, ptb[lo, 0:256], wtr, lo, TOP_S, TOP_TAPS,
           lambda dy, dx: Xv[lo, 1 + dy, dx + 1: dx + 257])
    _chain(nc, ptb[up, 0:256], wtr, up, BOT_S, BOT_TAPS,
           lambda dy, dx: Xv[up, 128 + dy, dx + 1: dx + 257])
    nc.scalar.activation(esb[:, 256:512], ptb[:, 0:256], AF.Identity,
                         bias=bias_t[:, B_TOPBOT:B_TOPBOT + 1])
    # corners: (0,0) d8 / (255,0) d2 at cols 512:514;
    #          (0,255) d6 / (255,255) d0 at cols 514:516
    pcn = pp.tile([128, 256], F32, tag="ps2")
    _chain(nc, pcn[lo, 0:2], wtr, lo, C8_S, C8_TAPS,
           lambda dy, dx: Xv[lo, 1 + dy, dx + 1: dx + 3])
    _chain(nc, pcn[up, 0:2], wtr, up, C2_S, C2_TAPS,
           lambda dy, dx: Xv[up, 128 + dy, dx + 1: dx + 3])
    _chain(nc, pcn[lo, 2:4], wtr, lo, C6_S, C6_TAPS,
           lambda dy, dx: Xv[lo, 1 + dy, 255 + dx: 257 + dx])
    _chain(nc, pcn[up, 2:4], wtr, up, C0_S, C0_TAPS,
           lambda dy, dx: Xv[up, 128 + dy, 255 + dx: 257 + dx])
    nc.scalar.activation(esb[:, 512:514], pcn[:, 0:2], AF.Identity,
                         bias=bias_t[:, B_C82:B_C82 + 1])
    nc.scalar.activation(esb[:, 514:516], pcn[:, 2:4], AF.Identity,
                         bias=bias_t[:, B_C60:B_C60 + 1])
    nc.sync.dma_start(edg_d[:], esb[:])


def _build():
    nc = bacc.Bacc("TRN2", target_bir_lowering=False, debug=False,
                   num_devices=NCORES)
    ip = nc.dram_tensor("img_prep", [128, XCOLS], BF16,
                        kind="ExternalInput").ap()
    wtm_d = nc.dram_tensor("wtm", [128, NWM * 64], BF16,
                           kind="ExternalInput").ap()
    wte_d = nc.dram_tensor("wte", [128, (NW - NWM) * 64], BF16,
                           kind="ExternalInput").ap()
    bias_d = nc.dram_tensor("bias", [128, NB], F32, kind="ExternalInput").ap()
    out_d = nc.dram_tensor("out", [2, 128, OCOLS], BF16,
                           kind="ExternalOutput").ap()
    edg_d = nc.dram_tensor("edges", [128, EOUT], F32,
                           kind="ExternalOutput").ap()

    lo, up = slice(0, 64), slice(64, 128)

    with tile.TileContext(nc) as tc:
        with (
            tc.tile_pool(name="const", bufs=1) as constp,
            tc.tile_pool(name="psmain", bufs=4, space="PSUM") as pp,
        ):
            wt = constp.tile([128, NW * 64], BF16)
            nc.sync.dma_start(wt[:, 0:NWM * 64], wtm_d[:])
            X = constp.tile([128, XCOLS], BF16)
            a0, b0 = XCHUNKS[0] * WPAD, XCHUNKS[1] * WPAD
            nc.sync.dma_start(X[:, a0:b0], ip[:, a0:b0])
            bias_t = constp.tile([128, NB], F32)
            nc.sync.dma_start(bias_t[:], bias_d[:])
            for ci in range(1, len(XCHUNKS) - 1):
                a, b = XCHUNKS[ci] * WPAD, XCHUNKS[ci + 1] * WPAD
                nc.sync.dma_start(X[:, a:b], ip[:, a:b])
            nc.sync.dma_start(wt[:, NWM * 64:], wte_d[:])
            wtr = wt[:]
            Xv = X[:].rearrange("p (t m) -> p t m", m=WPAD)

            olo = constp.tile([128, OCOLS], BF16)
            oup = constp.tile([128, OCOLS], BF16)

            # ---- dense interior conv (d=4) ----
            for g in range(NG):
                ps1 = pp.tile([128, 512], F32, tag="ps1")
                ps2 = pp.tile([128, 512], F32, tag="ps2")
                for k, (dy, dx) in enumerate(TAPS9):
                    st, sp = (k == 0), (k == 8)
                    for (ph, po, i) in ((lo, slice(0, 64), 4 * g),
                                        (up, slice(0, 64), 4 * g),
                                        (lo, slice(64, 128), 4 * g + 2),
                                        (up, slice(64, 128), 4 * g + 2)):
                        psd = (ps1 if ph == lo else ps2)
                        rhs = Xv[ph, i + 1 + dy: i + 3 + dy,
                                 dx + 1: dx + 257]
                        nc.tensor.matmul(
                            psd[po, :],
                            wtr[ph, ts(MAIN_S + k, 64)], rhs,
                            start=st, stop=sp, skip_group_check=True)
                # evacuate: bias add psum -> sbuf bf16; alternate engines
                blo = bias_t[:, B_MAIN:B_MAIN + 1]
                if g % 2 == 0:
                    nc.scalar.activation(olo[:, ts(g, 512)], ps1[:],
                                         AF.Identity, bias=blo)
                    nc.vector.tensor_scalar_add(oup[:, ts(g, 512)],
                                                ps2[:], blo)
                else:
                    nc.vector.tensor_scalar_add(olo[:, ts(g, 512)],
                                                ps1[:], blo)
                    nc.scalar.activation(oup[:, ts(g, 512)], ps2[:],
                                         AF.Identity, bias=blo)
                if g % OG == OG - 1:
                    a, b = (g - OG + 1) * 512, (g + 1) * 512
                    nc.sync.dma_start(out_d[0, :, a:b], olo[:, a:b])
                    nc.sync.dma_start(out_d[1, :, a:b], oup[:, a:b])
                if g == EDGE_AT - 1:
                    _edges(nc, pp, constp, wtr, bias_t, Xv, edg_d, lo, up)

    nc.compile()
    return nc


def _get_nc():
    if "nc" not in _CACHE:
        _CACHE["nc"] = _build()
    return _CACHE["nc"]


def _prep_img(imgc):
    """[64,256,256] f32 -> [128, XCOLS] whole-image padded bf16 layout."""
    ipv = np.zeros((2, 64, SLOTS, WPAD), BF)
    ipv[0, :, 1:130, 1:257] = imgc[:, 0:129, :]     # lo: rows -1..128
    ipv[1, :, 0:129, 1:257] = imgc[:, 127:256, :]   # up: rows 127..256
    return np.ascontiguousarray(ipv.reshape(128, XCOLS))


def _prep_wt(weights):
    wt = np.zeros((128, NW, 64), BF)
    for (d, taps), base in zip(W_GROUPS, _offs):
        for k, (dy, dx) in enumerate(taps):
            m = weights[d][:, :, dy + 1, dx + 1].T  # [cin, cout]
            wt[0:64, base + k] = m
            wt[64:128, base + k] = m
    return np.ascontiguousarray(wt.reshape(128, NW * 64))


def _prep_bias(bias):
    bs = np.zeros((128, NB), np.float32)
    for c, (dl, du) in enumerate(BIAS_PAIRS):
        bs[0:64, c] = bias[dl]
        bs[64:128, c] = bias[du]
    return bs


def _make_in_maps(img, weights, bias):
    img = np.asarray(img, np.float32)
    wt = _prep_wt(np.asarray(weights, np.float32))
    wtm = np.ascontiguousarray(wt[:, :NWM * 64])
    wte = np.ascontiguousarray(wt[:, NWM * 64:])
    bs = _prep_bias(np.asarray(bias, np.float32))
    return [{"img_prep": _prep_img(img[c]), "wtm": wtm, "wte": wte,
             "bias": bs}
            for c in range(NCORES)]


def _unprep_out(o, e):
    """Assemble [C,H,W] f32 from dense bf16 out + f32 edge overlay."""
    v = o.reshape(2, 2, 64, NG, 2, 256)  # h pg c g r w
    out = np.ascontiguousarray(
        v.transpose(2, 0, 3, 1, 4, 5).reshape(C, H, W)).astype(np.float32)
    Lv = np.concatenate([e[0:64, 0:128], e[64:128, 0:128]], axis=1)
    Rv = np.concatenate([e[0:64, 128:256], e[64:128, 128:256]], axis=1)
    out[:, 1:255, 0] = Lv[:, 1:255]
    out[:, 1:255, 255] = Rv[:, 1:255]
    out[:, 0, 1:255] = e[0:64, 257:511]
    out[:, 255, 1:255] = e[64:128, 257:511]
    out[:, 0, 0] = e[0:64, 512]
    out[:, 255, 0] = e[64:128, 512]
    out[:, 0, 255] = e[0:64, 515]
    out[:, 255, 255] = e[64:128, 515]
    return out


def kernel(img, weights, bias):
    nc = _get_nc()
    in_maps = _make_in_maps(img, weights, bias)
    res = run_bass_kernel_spmd(nc, in_maps, list(range(NCORES)))
    return np.stack([_unprep_out(res.results[c]["out"],
                                 res.results[c]["edges"])
                     for c in range(NCORES)])


# revision 12
# speedup vs baseline: 1.1605x; 1.0203x over previous
"""bf16 4-way PE-quadrant conv, whole-image SBUF residency, bf16 output.

Layout: the full image lives in one SBUF tile [128, 130, 258] bf16 per core.
Partitions 0-63 (lo) hold channels for image rows -1..128 (slot 0 dummy);
partitions 64-127 (up) hold rows 127..256 (slot 129 dummy). Columns padded by
one on each side. Border outputs consume dummy/pad garbage but are overlaid
from the separately-computed edge tensor on the host.

Main conv (d=4): per 4-row group, 9 taps x 4 concurrent 64x64 bf16 quadrant
matmuls (lo/up half x PSUM col half). Output accumulates into one persistent
bf16 tile DMA'd out every OG groups. Edge pixels are computed mid-loop
directly from the resident image, with taps interleaved across independent
chains so different PE quadrants stream concurrently.

DMA ordering: HW queues deliver packets strictly in dispatch order, so small
tensors needed early (wtm, bias) are dispatched before the bulk image chunks.
A block of dummy warmup matmuls on scratch data ramps the PE clock while the
first image chunk is still in flight.
"""

import ml_dtypes
import numpy as np

import concourse.bacc as bacc
import concourse.mybir as mybir
import concourse.tile as tile
from concourse.bass import ts
from concourse.bass_utils import run_bass_kernel_spmd

B, C, H, W = 8, 64, 256, 256
NCORES = 8
H2 = H // 2          # rows per partition-half
SLOTS = H2 + 2       # 130
WPAD = W + 2         # 258
XCOLS = SLOTS * WPAD
NG = H2 // 4         # 32 groups of 4 rows per half
OCOLS = NG * 512     # 16384 output cols per half
F32 = mybir.dt.float32
BF16 = mybir.dt.bfloat16
AF = mybir.ActivationFunctionType
BF = ml_dtypes.bfloat16

TAPS9 = [(dy, dx) for dy in (-1, 0, 1) for dx in (-1, 0, 1)]
TOP_TAPS = [(dy, dx) for dy in (0, 1) for dx in (-1, 0, 1)]      # d=7 row 0
BOT_TAPS = [(dy, dx) for dy in (-1, 0) for dx in (-1, 0, 1)]     # d=1 row 255
LEFT_TAPS = [(dy, dx) for dy in (-1, 0, 1) for dx in (0, 1)]     # d=5 col 0
RIGHT_TAPS = [(dy, dx) for dy in (-1, 0, 1) for dx in (-1, 0)]   # d=3 col 255
C6_TAPS = [(dy, dx) for dy in (0, 1) for dx in (-1, 0)]          # d=6 (0,255)
C2_TAPS = [(dy, dx) for dy in (-1, 0) for dx in (0, 1)]          # d=2 (255,0)
C8_TAPS = [(dy, dx) for dy in (0, 1) for dx in (0, 1)]           # d=8 (0,0)
C0_TAPS = [(dy, dx) for dy in (-1, 0) for dx in (-1, 0)]         # d=0 (255,255)

W_GROUPS = [
    (4, TAPS9), (7, TOP_TAPS), (1, BOT_TAPS), (5, LEFT_TAPS),
    (3, RIGHT_TAPS), (6, C6_TAPS), (2, C2_TAPS), (8, C8_TAPS), (0, C0_TAPS),
]
_offs = []
_acc = 0
for _d, _taps in W_GROUPS:
    _offs.append(_acc)
    _acc += len(_taps)
(MAIN_S, TOP_S, BOT_S, LEFT_S, RIGHT_S, C6_S, C2_S, C8_S, C0_S) = _offs
NW = _acc  # 49
NWM = 9    # main taps at the front of wt
# bias tile [128, NB]: column -> (value on partitions 0-63, on 64-127)
BIAS_PAIRS = [(4, 4), (5, 5), (3, 3), (7, 1), (8, 2), (6, 0)]
B_MAIN, B_LEFT, B_RIGHT, B_TOPBOT, B_C82, B_C60 = range(6)
NB = 6

EOUT = 516  # edges_out: [0:128] left, [128:256] right, [256:512] top|bottom,
            # [512:514] corner j01 (d8|d2), [514:516] corner j254/255 (d6|d0)

# input DMA chunk boundaries (slot index); first chunk small for fast start
XCHUNKS = [0, 10, 18, 46, 74, 102, SLOTS]
# output DMA every OG groups
OG = 2
# issue the edge computation after this many main-loop groups (image fully
# resident by then; keeps edge DMA latency off the kernel tail)
EDGE_AT = 18
# dummy matmul groups (structured exactly like main-loop groups) issued
# before the main loop to ramp the PE clock while the first image chunk is
# still in flight
WARMUP_GROUPS = 3

_CACHE = {}


def _interleaved_chains(nc, chains):
    """Issue accumulation chains with taps round-robined across chains so
    independent PE quadrants stream concurrently. Each chain is
    (psd, wslice, slot0, taps, rhs_fn)."""
    nmax = max(len(c[3]) for c in chains)
    for k in range(nmax):
        for (psd, wslice, slot0, taps, rhs_fn) in chains:
            if k >= len(taps):
                continue
            dy, dx = taps[k]
            nc.tensor.matmul(psd, wslice[:, ts(slot0 + k, 64)],
                             rhs_fn(dy, dx),
                             start=(k == 0), stop=(k == len(taps) - 1),
                             skip_group_check=True)


def _edges(nc, pp, constp, wtr, bias_t, Xv, edg_d, lo, up):
    """Border-pixel conv chains, reading the resident image tile."""
    esb = constp.tile([128, EOUT], F32)
    # side columns: 4 chains on quadrants (h0,c0) (h64,c64) (h0,c0) (h64,c64)
    pside = pp.tile([128, 256], F32, tag="ps1")
    pside2 = pp.tile([128, 256], F32, tag="ps2")
    _interleaved_chains(nc, [
        (pside[lo, 0:128], wtr[lo], LEFT_S, LEFT_TAPS,
         lambda dy, dx: Xv[lo, 1 + dy: 129 + dy, dx + 1]),
        (pside[up, 0:128], wtr[up], LEFT_S, LEFT_TAPS,
         lambda dy, dx: Xv[up, 1 + dy: 129 + dy, dx + 1]),
        (pside2[lo, 0:128], wtr[lo], RIGHT_S, RIGHT_TAPS,
         lambda dy, dx: Xv[lo, 1 + dy: 129 + dy, dx + 256]),
        (pside2[up, 0:128], wtr[up], RIGHT_S, RIGHT_TAPS,
         lambda dy, dx: Xv[up, 1 + dy: 129 + dy, dx + 256]),
    ])
    nc.scalar.activation(esb[:, 0:128], pside[:, 0:128], AF.Identity,
                         bias=bias_t[:, B_LEFT:B_LEFT + 1])
    nc.scalar.activation(esb[:, 128:256], pside2[:, 0:128], AF.Identity,
                         bias=bias_t[:, B_RIGHT:B_RIGHT + 1])
    # top row (d=7) on lo / bottom row (d=1) on up
    ptb = pp.tile([128, 256], F32, tag="ps1")
    _interleaved_chains(nc, [
        (ptb[lo, 0:256], wtr[lo], TOP_S, TOP_TAPS,
         lambda dy, dx: Xv[lo, 1 + dy, dx + 1: dx + 257]),
        (ptb[up, 0:256], wtr[up], BOT_S, BOT_TAPS,
         lambda dy, dx: Xv[up, 128 + dy, dx + 1: dx + 257]),
    ])
    nc.scalar.activation(esb[:, 256:512], ptb[:, 0:256], AF.Identity,
                         bias=bias_t[:, B_TOPBOT:B_TOPBOT + 1])
    # corners: (0,0) d8 / (255,0) d2 at cols 512:514;
    #          (0,255) d6 / (255,255) d0 at cols 514:516.
    # C8/C2 and C6/C0 share partitions, so interleaved accumulation epochs
    # must live in distinct PSUM banks (pcn vs pcn2).
    pcn = pp.tile([128, 256], F32, tag="ps2")
    pcn2 = pp.tile([128, 256], F32, tag="ps1")
    _interleaved_chains(nc, [
        (pcn[lo, 0:2], wtr[lo], C8_S, C8_TAPS,
         lambda dy, dx: Xv[lo, 1 + dy, dx + 1: dx + 3]),
        (pcn[up, 0:2], wtr[up], C2_S, C2_TAPS,
         lambda dy, dx: Xv[up, 128 + dy, dx + 1: dx + 3]),
        (pcn2[lo, 2:4], wtr[lo], C6_S, C6_TAPS,
         lambda dy, dx: Xv[lo, 1 + dy, 255 + dx: 257 + dx]),
        (pcn2[up, 2:4], wtr[up], C0_S, C0_TAPS,
         lambda dy, dx: Xv[up, 128 + dy, 255 + dx: 257 + dx]),
    ])
    nc.scalar.activation(esb[:, 512:514], pcn[:, 0:2], AF.Identity,
                         bias=bias_t[:, B_C82:B_C82 + 1])
    nc.scalar.activation(esb[:, 514:516], pcn2[:, 2:4], AF.Identity,
                         bias=bias_t[:, B_C60:B_C60 + 1])
    nc.sync.dma_start(edg_d[:], esb[:])


def _build():
    nc = bacc.Bacc("TRN2", target_bir_lowering=False, debug=False,
                   num_devices=NCORES)
    ip = nc.dram_tensor("img_prep", [128, XCOLS], BF16,
                        kind="ExternalInput").ap()
    wtm_d = nc.dram_tensor("wtm", [128, NWM * 64], BF16,
                           kind="ExternalInput").ap()
    wte_d = nc.dram_tensor("wte", [128, (NW - NWM) * 64], BF16,
                           kind="ExternalInput").ap()
    bias_d = nc.dram_tensor("bias", [128, NB], F32, kind="ExternalInput").ap()
    out_d = nc.dram_tensor("out", [128, 2, OCOLS], BF16,
                           kind="ExternalOutput").ap()
    edg_d = nc.dram_tensor("edges", [128, EOUT], F32,
                           kind="ExternalOutput").ap()

    lo, up = slice(0, 64), slice(64, 128)

    with tile.TileContext(nc) as tc:
        with (
            tc.tile_pool(name="const", bufs=1) as constp,
            tc.tile_pool(name="psmain", bufs=4, space="PSUM") as pp,
        ):
            # scratch for PE warmup (memset so CoreSim sees initialized data)
            if WARMUP_GROUPS:
                scr = constp.tile([128, 512], BF16)
                nc.vector.memset(scr[:], 0.0)

            wt = constp.tile([128, NW * 64], BF16)
            nc.sync.dma_start(wt[:, 0:NWM * 64], wtm_d[:])
            X = constp.tile([128, XCOLS], BF16)
            a0, b0 = XCHUNKS[0] * WPAD, XCHUNKS[1] * WPAD
            nc.sync.dma_start(X[:, a0:b0], ip[:, a0:b0])
            bias_t = constp.tile([128, NB], F32)
            nc.sync.dma_start(bias_t[:], bias_d[:])
            for ci in range(1, len(XCHUNKS) - 1):
                a, b = XCHUNKS[ci] * WPAD, XCHUNKS[ci + 1] * WPAD
                nc.sync.dma_start(X[:, a:b], ip[:, a:b])
            nc.sync.dma_start(wt[:, NWM * 64:], wte_d[:])
            wtr = wt[:]
            Xv = X[:].rearrange("p (t m) -> p t m", m=WPAD)

            # PE clock warmup on scratch data (no DMA dependencies);
            # structured exactly like a main-loop group so PSUM accumulation
            # epochs per bank+partition stay sequential.
            for _ in range(WARMUP_GROUPS):
                pw1 = pp.tile([128, 512], F32, tag="ps1")
                pw2 = pp.tile([128, 512], F32, tag="ps2")
                for k in range(9):
                    st, sp = (k == 0), (k == 8)
                    for (ph, po) in ((lo, slice(0, 64)), (up, slice(0, 64)),
                                     (lo, slice(64, 128)),
                                     (up, slice(64, 128))):
                        psd = (pw1 if ph == lo else pw2)
                        nc.tensor.matmul(psd[po, :], scr[ph, 0:64],
                                         scr[ph, :], start=st, stop=sp,
                                         skip_group_check=True)

            ot = constp.tile([128, 2 * OCOLS], BF16)
            otv = ot[:].rearrange("p (h c) -> p h c", h=2)

            # ---- dense interior conv (d=4) ----
            for g in range(NG):
                ps1 = pp.tile([128, 512], F32, tag="ps1")
                ps2 = pp.tile([128, 512], F32, tag="ps2")
                for k, (dy, dx) in enumerate(TAPS9):
                    st, sp = (k == 0), (k == 8)
                    for (ph, po, i) in ((lo, slice(0, 64), 4 * g),
                                        (up, slice(0, 64), 4 * g),
                                        (lo, slice(64, 128), 4 * g + 2),
                                        (up, slice(64, 128), 4 * g + 2)):
                        psd = (ps1 if ph == lo else ps2)
                        rhs = Xv[ph, i + 1 + dy: i + 3 + dy,
                                 dx + 1: dx + 257]
                        nc.tensor.matmul(
                            psd[po, :],
                            wtr[ph, ts(MAIN_S + k, 64)], rhs,
                            start=st, stop=sp, skip_group_check=True)
                # evacuate: bias add psum -> sbuf bf16; alternate engines
                blo = bias_t[:, B_MAIN:B_MAIN + 1]
                if g % 2 == 0:
                    nc.scalar.activation(ot[:, ts(g, 512)], ps1[:],
                                         AF.Identity, bias=blo)
                    nc.vector.tensor_scalar_add(ot[:, OCOLS + 512 * g:
                                                   OCOLS + 512 * (g + 1)],
                                                ps2[:], blo)
                else:
                    nc.vector.tensor_scalar_add(ot[:, ts(g, 512)],
                                                ps1[:], blo)
                    nc.scalar.activation(ot[:, OCOLS + 512 * g:
                                            OCOLS + 512 * (g + 1)], ps2[:],
                                         AF.Identity, bias=blo)
                if g % OG == OG - 1:
                    a, b = (g - OG + 1) * 512, (g + 1) * 512
                    nc.sync.dma_start(out_d[:, :, a:b], otv[:, :, a:b])
                if g == EDGE_AT - 1:
                    _edges(nc, pp, constp, wtr, bias_t, Xv, edg_d, lo, up)

    nc.compile()
    return nc


def _get_nc():
    if "nc" not in _CACHE:
        _CACHE["nc"] = _build()
    return _CACHE["nc"]


def _prep_img(imgc):
    """[64,256,256] f32 -> [128, XCOLS] whole-image padded bf16 layout."""
    ipv = np.zeros((2, 64, SLOTS, WPAD), BF)
    ipv[0, :, 1:130, 1:257] = imgc[:, 0:129, :]     # lo: rows -1..128
    ipv[1, :, 0:129, 1:257] = imgc[:, 127:256, :]   # up: rows 127..256
    return np.ascontiguousarray(ipv.reshape(128, XCOLS))


def _prep_wt(weights):
    wt = np.zeros((128, NW, 64), BF)
    for (d, taps), base in zip(W_GROUPS, _offs):
        for k, (dy, dx) in enumerate(taps):
            m = weights[d][:, :, dy + 1, dx + 1].T  # [cin, cout]
            wt[0:64, base + k] = m
            wt[64:128, base + k] = m
    return np.ascontiguousarray(wt.reshape(128, NW * 64))


def _prep_bias(bias):
    bs = np.zeros((128, NB), np.float32)
    for c, (dl, du) in enumerate(BIAS_PAIRS):
        bs[0:64, c] = bias[dl]
        bs[64:128, c] = bias[du]
    return bs


def _make_in_maps(img, weights, bias):
    img = np.asarray(img, np.float32)
    wt = _prep_wt(np.asarray(weights, np.float32))
    wtm = np.ascontiguousarray(wt[:, :NWM * 64])
    wte = np.ascontiguousarray(wt[:, NWM * 64:])
    bs = _prep_bias(np.asarray(bias, np.float32))
    return [{"img_prep": _prep_img(img[c]), "wtm": wtm, "wte": wte,
             "bias": bs}
            for c in range(NCORES)]


def _unprep_out(o, e):
    """Assemble [C,H,W] f32 from dense bf16 out + f32 edge overlay."""
    v = o.reshape(2, 64, 2, NG, 2, 256)  # pg c h g r w
    out = np.ascontiguousarray(
        v.transpose(1, 2, 3, 0, 4, 5).reshape(C, H, W)).astype(np.float32)
    Lv = np.concatenate([e[0:64, 0:128], e[64:128, 0:128]], axis=1)
    Rv = np.concatenate([e[0:64, 128:256], e[64:128, 128:256]], axis=1)
    out[:, 1:255, 0] = Lv[:, 1:255]
    out[:, 1:255, 255] = Rv[:, 1:255]
    out[:, 0, 1:255] = e[0:64, 257:511]
    out[:, 255, 1:255] = e[64:128, 257:511]
    out[:, 0, 0] = e[0:64, 512]
    out[:, 255, 0] = e[64:128, 512]
    out[:, 0, 255] = e[0:64, 515]
    out[:, 255, 255] = e[64:128, 515]
    return out


def kernel(img, weights, bias):
    nc = _get_nc()
    in_maps = _make_in_maps(img, weights, bias)
    res = run_bass_kernel_spmd(nc, in_maps, list(range(NCORES)))
    return np.stack([_unprep_out(res.results[c]["out"],
                                 res.results[c]["edges"])
                     for c in range(NCORES)])


# revision 15
# speedup vs baseline: 1.1681x; 1.0066x over previous
"""bf16 4-way PE-quadrant conv, whole-image SBUF residency, bf16 output.

Layout: the full image lives in one SBUF tile [128, 130, 258] bf16 per core.
Partitions 0-63 (lo) hold channels for image rows -1..128 (slot 0 dummy);
partitions 64-127 (up) hold rows 127..256 (slot 129 dummy). Columns padded by
one on each side. Border outputs consume dummy/pad garbage but are overlaid
from the separately-computed edge tensor on the host.

Main conv (d=4): per 4-row group, 9 taps x 4 concurrent 64x64 bf16 quadrant
matmuls (lo/up half x PSUM col half). Output accumulates into one persistent
bf16 tile DMA'd out every OG groups. Edge pixels are computed mid-loop
directly from the resident image, with taps interleaved across independent
chains so different PE quadrants stream concurrently.

DMA ordering: HW queues deliver packets strictly in dispatch order, so small
tensors needed early (wtm, bias) are dispatched before the bulk image chunks.
A block of dummy warmup matmuls on scratch data ramps the PE clock while the
first image chunk is still in flight.
"""

import ml_dtypes
import numpy as np

import concourse.bacc as bacc
import concourse.mybir as mybir
import concourse.tile as tile
from concourse.bass import ts
from concourse.bass_utils import run_bass_kernel_spmd

B, C, H, W = 8, 64, 256, 256
NCORES = 8
H2 = H // 2          # rows per partition-half
SLOTS = H2 + 2       # 130
WPAD = W + 2         # 258
XCOLS = SLOTS * WPAD
NG = H2 // 4         # 32 groups of 4 rows per half
OCOLS = NG * 512     # 16384 output cols per half
F32 = mybir.dt.float32
BF16 = mybir.dt.bfloat16
AF = mybir.ActivationFunctionType
BF = ml_dtypes.bfloat16

TAPS9 = [(dy, dx) for dy in (-1, 0, 1) for dx in (-1, 0, 1)]
TOP_TAPS = [(dy, dx) for dy in (0, 1) for dx in (-1, 0, 1)]      # d=7 row 0
BOT_TAPS = [(dy, dx) for dy in (-1, 0) for dx in (-1, 0, 1)]     # d=1 row 255
LEFT_TAPS = [(dy, dx) for dy in (-1, 0, 1) for dx in (0, 1)]     # d=5 col 0
RIGHT_TAPS = [(dy, dx) for dy in (-1, 0, 1) for dx in (-1, 0)]   # d=3 col 255
C6_TAPS = [(dy, dx) for dy in (0, 1) for dx in (-1, 0)]          # d=6 (0,255)
C2_TAPS = [(dy, dx) for dy in (-1, 0) for dx in (0, 1)]          # d=2 (255,0)
C8_TAPS = [(dy, dx) for dy in (0, 1) for dx in (0, 1)]           # d=8 (0,0)
C0_TAPS = [(dy, dx) for dy in (-1, 0) for dx in (-1, 0)]         # d=0 (255,255)

W_GROUPS = [
    (4, TAPS9), (7, TOP_TAPS), (1, BOT_TAPS), (5, LEFT_TAPS),
    (3, RIGHT_TAPS), (6, C6_TAPS), (2, C2_TAPS), (8, C8_TAPS), (0, C0_TAPS),
]
_offs = []
_acc = 0
for _d, _taps in W_GROUPS:
    _offs.append(_acc)
    _acc += len(_taps)
(MAIN_S, TOP_S, BOT_S, LEFT_S, RIGHT_S, C6_S, C2_S, C8_S, C0_S) = _offs
NW = _acc  # 49
NWM = 9    # main taps at the front of wt
# bias tile [128, NB]: column -> (value on partitions 0-63, on 64-127)
BIAS_PAIRS = [(4, 4), (5, 5), (3, 3), (7, 1), (8, 2), (6, 0)]
B_MAIN, B_LEFT, B_RIGHT, B_TOPBOT, B_C82, B_C60 = range(6)
NB = 6

EOUT = 516  # edges_out: [0:128] left, [128:256] right, [256:512] top|bottom,
            # [512:514] corner j01 (d8|d2), [514:516] corner j254/255 (d6|d0)

# input DMA chunk boundaries (slot index); first chunk small for fast start
XCHUNKS = [0, 10, 18, 46, 74, 102, SLOTS]
# output DMA every OG groups
OG = 2
# issue the edge computation after this many main-loop groups (image fully
# resident by then; keeps edge DMA latency off the kernel tail)
EDGE_AT = 18
# dummy matmul groups (structured exactly like main-loop groups) issued
# before the main loop to ramp the PE clock while the first image chunk is
# still in flight
WARMUP_GROUPS = 3

_CACHE = {}


def _interleaved_chains(nc, chains):
    """Issue accumulation chains with taps round-robined across chains so
    independent PE quadrants stream concurrently. Each chain is
    (psd, wslice, slot0, taps, rhs_fn)."""
    nmax = max(len(c[3]) for c in chains)
    for k in range(nmax):
        for (psd, wslice, slot0, taps, rhs_fn) in chains:
            if k >= len(taps):
                continue
            dy, dx = taps[k]
            nc.tensor.matmul(psd, wslice[:, ts(slot0 + k, 64)],
                             rhs_fn(dy, dx),
                             start=(k == 0), stop=(k == len(taps) - 1),
                             skip_group_check=True)


def _edges(nc, pp, constp, wtr, bias_t, Xv, edg_d, lo, up):
    """Border-pixel conv chains, reading the resident image tile."""
    esb = constp.tile([128, EOUT], F32)
    # side columns: 4 chains on quadrants (h0,c0) (h64,c64) (h0,c0) (h64,c64)
    pside = pp.tile([128, 256], F32, tag="ps1")
    pside2 = pp.tile([128, 256], F32, tag="ps2")
    _interleaved_chains(nc, [
        (pside[lo, 0:128], wtr[lo], LEFT_S, LEFT_TAPS,
         lambda dy, dx: Xv[lo, 1 + dy: 129 + dy, dx + 1]),
        (pside[up, 0:128], wtr[up], LEFT_S, LEFT_TAPS,
         lambda dy, dx: Xv[up, 1 + dy: 129 + dy, dx + 1]),
        (pside2[lo, 0:128], wtr[lo], RIGHT_S, RIGHT_TAPS,
         lambda dy, dx: Xv[lo, 1 + dy: 129 + dy, dx + 256]),
        (pside2[up, 0:128], wtr[up], RIGHT_S, RIGHT_TAPS,
         lambda dy, dx: Xv[up, 1 + dy: 129 + dy, dx + 256]),
    ])
    nc.scalar.activation(esb[:, 0:128], pside[:, 0:128], AF.Identity,
                         bias=bias_t[:, B_LEFT:B_LEFT + 1])
    nc.scalar.activation(esb[:, 128:256], pside2[:, 0:128], AF.Identity,
                         bias=bias_t[:, B_RIGHT:B_RIGHT + 1])
    # top row (d=7) on lo / bottom row (d=1) on up
    ptb = pp.tile([128, 256], F32, tag="ps1")
    _interleaved_chains(nc, [
        (ptb[lo, 0:256], wtr[lo], TOP_S, TOP_TAPS,
         lambda dy, dx: Xv[lo, 1 + dy, dx + 1: dx + 257]),
        (ptb[up, 0:256], wtr[up], BOT_S, BOT_TAPS,
         lambda dy, dx: Xv[up, 128 + dy, dx + 1: dx + 257]),
    ])
    nc.scalar.activation(esb[:, 256:512], ptb[:, 0:256], AF.Identity,
                         bias=bias_t[:, B_TOPBOT:B_TOPBOT + 1])
    # corners: (0,0) d8 / (255,0) d2 at cols 512:514;
    #          (0,255) d6 / (255,255) d0 at cols 514:516.
    # C8/C2 and C6/C0 share partitions, so interleaved accumulation epochs
    # must live in distinct PSUM banks (pcn vs pcn2).
    pcn = pp.tile([128, 256], F32, tag="ps2")
    pcn2 = pp.tile([128, 256], F32, tag="ps1")
    _interleaved_chains(nc, [
        (pcn[lo, 0:2], wtr[lo], C8_S, C8_TAPS,
         lambda dy, dx: Xv[lo, 1 + dy, dx + 1: dx + 3]),
        (pcn[up, 0:2], wtr[up], C2_S, C2_TAPS,
         lambda dy, dx: Xv[up, 128 + dy, dx + 1: dx + 3]),
        (pcn2[lo, 2:4], wtr[lo], C6_S, C6_TAPS,
         lambda dy, dx: Xv[lo, 1 + dy, 255 + dx: 257 + dx]),
        (pcn2[up, 2:4], wtr[up], C0_S, C0_TAPS,
         lambda dy, dx: Xv[up, 128 + dy, 255 + dx: 257 + dx]),
    ])
    nc.scalar.activation(esb[:, 512:514], pcn[:, 0:2], AF.Identity,
                         bias=bias_t[:, B_C82:B_C82 + 1])
    nc.scalar.activation(esb[:, 514:516], pcn2[:, 2:4], AF.Identity,
                         bias=bias_t[:, B_C60:B_C60 + 1])
    nc.sync.dma_start(edg_d[:], esb[:])


def _build():
    nc = bacc.Bacc("TRN2", target_bir_lowering=False, debug=False,
                   num_devices=NCORES)
    ip = nc.dram_tensor("img_prep", [128, XCOLS], BF16,
                        kind="ExternalInput").ap()
    wtm_d = nc.dram_tensor("wtm", [128, NWM * 64], BF16,
                           kind="ExternalInput").ap()
    wte_d = nc.dram_tensor("wte", [128, (NW - NWM) * 64], BF16,
                           kind="ExternalInput").ap()
    bias_d = nc.dram_tensor("bias", [128, NB], F32, kind="ExternalInput").ap()
    out_d = nc.dram_tensor("out", [128, 2, OCOLS], BF16,
                           kind="ExternalOutput").ap()
    edg_d = nc.dram_tensor("edges", [128, EOUT], F32,
                           kind="ExternalOutput").ap()

    lo, up = slice(0, 64), slice(64, 128)

    with tile.TileContext(nc) as tc:
        with (
            tc.tile_pool(name="const", bufs=1) as constp,
            tc.tile_pool(name="psmain", bufs=4, space="PSUM") as pp,
        ):
            # scratch for PE warmup (memset: the scheduler requires tiles to
            # be written before read; warmup then depends only on this)
            if WARMUP_GROUPS:
                scr = constp.tile([128, 512], BF16)
                nc.vector.memset(scr[:], 0.0)

            wt = constp.tile([128, NW * 64], BF16)
            nc.sync.dma_start(wt[:, 0:NWM * 64], wtm_d[:])
            X = constp.tile([128, XCOLS], BF16)
            a0, b0 = XCHUNKS[0] * WPAD, XCHUNKS[1] * WPAD
            nc.sync.dma_start(X[:, a0:b0], ip[:, a0:b0])
            bias_t = constp.tile([128, NB], F32)
            nc.sync.dma_start(bias_t[:], bias_d[:])
            for ci in range(1, len(XCHUNKS) - 1):
                a, b = XCHUNKS[ci] * WPAD, XCHUNKS[ci + 1] * WPAD
                nc.sync.dma_start(X[:, a:b], ip[:, a:b])
            nc.sync.dma_start(wt[:, NWM * 64:], wte_d[:])
            wtr = wt[:]
            Xv = X[:].rearrange("p (t m) -> p t m", m=WPAD)

            # PE clock warmup on scratch data (no DMA dependencies);
            # structured exactly like a main-loop group so PSUM accumulation
            # epochs per bank+partition stay sequential.
            for _ in range(WARMUP_GROUPS):
                pw1 = pp.tile([128, 512], F32, tag="ps1")
                pw2 = pp.tile([128, 512], F32, tag="ps2")
                for k in range(9):
                    st, sp = (k == 0), (k == 8)
                    for (ph, po) in ((lo, slice(0, 64)), (up, slice(0, 64)),
                                     (lo, slice(64, 128)),
                                     (up, slice(64, 128))):
                        psd = (pw1 if ph == lo else pw2)
                        nc.tensor.matmul(psd[po, :], scr[ph, 0:64],
                                         scr[ph, :], start=st, stop=sp,
                                         skip_group_check=True)

            ot = constp.tile([128, 2 * OCOLS], BF16)
            otv = ot[:].rearrange("p (h c) -> p h c", h=2)

            # ---- dense interior conv (d=4) ----
            for g in range(NG):
                ps1 = pp.tile([128, 512], F32, tag="ps1")
                ps2 = pp.tile([128, 512], F32, tag="ps2")
                for k, (dy, dx) in enumerate(TAPS9):
                    st, sp = (k == 0), (k == 8)
                    for (ph, po, i) in ((lo, slice(0, 64), 4 * g),
                                        (up, slice(0, 64), 4 * g),
                                        (lo, slice(64, 128), 4 * g + 2),
                                        (up, slice(64, 128), 4 * g + 2)):
                        psd = (ps1 if ph == lo else ps2)
                        rhs = Xv[ph, i + 1 + dy: i + 3 + dy,
                                 dx + 1: dx + 257]
                        nc.tensor.matmul(
                            psd[po, :],
                            wtr[ph, ts(MAIN_S + k, 64)], rhs,
                            start=st, stop=sp, skip_group_check=True)
                # evacuate: bias add psum -> sbuf bf16; alternate engines
                blo = bias_t[:, B_MAIN:B_MAIN + 1]
                if g % 2 == 0:
                    nc.scalar.activation(ot[:, ts(g, 512)], ps1[:],
                                         AF.Identity, bias=blo)
                    nc.vector.tensor_scalar_add(ot[:, OCOLS + 512 * g:
                                                   OCOLS + 512 * (g + 1)],
                                                ps2[:], blo)
                else:
                    nc.vector.tensor_scalar_add(ot[:, ts(g, 512)],
                                                ps1[:], blo)
                    nc.scalar.activation(ot[:, OCOLS + 512 * g:
                                            OCOLS + 512 * (g + 1)], ps2[:],
                                         AF.Identity, bias=blo)
                if g >= NG - 2:
                    # final groups go out singly to shorten the kernel tail
                    a, b = g * 512, (g + 1) * 512
                    nc.sync.dma_start(out_d[:, :, a:b], otv[:, :, a:b])
                elif g % OG == OG - 1:
                    a, b = (g - OG + 1) * 512, (g + 1) * 512
                    nc.sync.dma_start(out_d[:, :, a:b], otv[:, :, a:b])
                if g == EDGE_AT - 1:
                    _edges(nc, pp, constp, wtr, bias_t, Xv, edg_d, lo, up)

    nc.compile()
    return nc


def _get_nc():
    if "nc" not in _CACHE:
        _CACHE["nc"] = _build()
    return _CACHE["nc"]


def _prep_img(imgc):
    """[64,256,256] f32 -> [128, XCOLS] whole-image padded bf16 layout."""
    ipv = np.zeros((2, 64, SLOTS, WPAD), BF)
    ipv[0, :, 1:130, 1:257] = imgc[:, 0:129, :]     # lo: rows -1..128
    ipv[1, :, 0:129, 1:257] = imgc[:, 127:256, :]   # up: rows 127..256
    return np.ascontiguousarray(ipv.reshape(128, XCOLS))


def _prep_wt(weights):
    wt = np.zeros((128, NW, 64), BF)
    for (d, taps), base in zip(W_GROUPS, _offs):
        for k, (dy, dx) in enumerate(taps):
            m = weights[d][:, :, dy + 1, dx + 1].T  # [cin, cout]
            wt[0:64, base + k] = m
            wt[64:128, base + k] = m
    return np.ascontiguousarray(wt.reshape(128, NW * 64))


def _prep_bias(bias):
    bs = np.zeros((128, NB), np.float32)
    for c, (dl, du) in enumerate(BIAS_PAIRS):
        bs[0:64, c] = bias[dl]
        bs[64:128, c] = bias[du]
    return bs


def _make_in_maps(img, weights, bias):
    img = np.asarray(img, np.float32)
    wt = _prep_wt(np.asarray(weights, np.float32))
    wtm = np.ascontiguousarray(wt[:, :NWM * 64])
    wte = np.ascontiguousarray(wt[:, NWM * 64:])
    bs = _prep_bias(np.asarray(bias, np.float32))
    return [{"img_prep": _prep_img(img[c]), "wtm": wtm, "wte": wte,
             "bias": bs}
            for c in range(NCORES)]


def _unprep_out(o, e):
    """Assemble [C,H,W] f32 from dense bf16 out + f32 edge overlay."""
    v = o.reshape(2, 64, 2, NG, 2, 256)  # pg c h g r w
    out = np.ascontiguousarray(
        v.transpose(1, 2, 3, 0, 4, 5).reshape(C, H, W)).astype(np.float32)
    Lv = np.concatenate([e[0:64, 0:128], e[64:128, 0:128]], axis=1)
    Rv = np.concatenate([e[0:64, 128:256], e[64:128, 128:256]], axis=1)
    out[:, 1:255, 0] = Lv[:, 1:255]
    out[:, 1:255, 255] = Rv[:, 1:255]
    out[:, 0, 1:255] = e[0:64, 257:511]
    out[:, 255, 1:255] = e[64:128, 257:511]
    out[:, 0, 0] = e[0:64, 512]
    out[:, 255, 0] = e[64:128, 512]
    out[:, 0, 255] = e[0:64, 515]
    out[:, 255, 255] = e[64:128, 515]
    return out


def kernel(img, weights, bias):
    nc = _get_nc()
    in_maps = _make_in_maps(img, weights, bias)
    res = run_bass_kernel_spmd(nc, in_maps, list(range(NCORES)))
    return np.stack([_unprep_out(res.results[c]["out"],
                                 res.results[c]["edges"])
                     for c in range(NCORES)])


# revision 16
# speedup vs baseline: 1.1969x; 1.0247x over previous
"""bf16 4-way PE-quadrant conv, whole-image SBUF residency, bf16 output.

Layout: the full image lives in one SBUF tile [128, 130, 258] bf16 per core.
Partitions 0-63 (lo) hold channels for image rows -1..128 (slot 0 dummy);
partitions 64-127 (up) hold rows 127..256 (slot 129 dummy). Columns padded by
one on each side. Border outputs consume dummy/pad garbage but are overlaid
from the separately-computed edge tensor on the host.

Main conv (d=4): per 4-row group, 9 taps x 4 concurrent 64x64 bf16 quadrant
matmuls (lo/up half x PSUM col half). Output accumulates into one persistent
bf16 tile DMA'd out every OG groups. Edge pixels are computed mid-loop
directly from the resident image, with taps interleaved across independent
chains so different PE quadrants stream concurrently.

DMA ordering: HW queues deliver packets strictly in dispatch order, so small
tensors needed early (wtm, bias) are dispatched before the bulk image chunks.
A block of dummy warmup matmuls on scratch data ramps the PE clock while the
first image chunk is still in flight.
"""

import ml_dtypes
import numpy as np

import concourse.bacc as bacc
import concourse.mybir as mybir
import concourse.tile as tile
from concourse.bass import ts
from concourse.bass_utils import run_bass_kernel_spmd

B, C, H, W = 8, 64, 256, 256
NCORES = 8
H2 = H // 2          # rows per partition-half
SLOTS = H2 + 2       # 130
WPAD = W + 2         # 258
XCOLS = SLOTS * WPAD
NG = H2 // 4         # 32 groups of 4 rows per half
OCOLS = NG * 512     # 16384 output cols per half
F32 = mybir.dt.float32
BF16 = mybir.dt.bfloat16
AF = mybir.ActivationFunctionType
BF = ml_dtypes.bfloat16

TAPS9 = [(dy, dx) for dy in (-1, 0, 1) for dx in (-1, 0, 1)]
TOP_TAPS = [(dy, dx) for dy in (0, 1) for dx in (-1, 0, 1)]      # d=7 row 0
BOT_TAPS = [(dy, dx) for dy in (-1, 0) for dx in (-1, 0, 1)]     # d=1 row 255
LEFT_TAPS = [(dy, dx) for dy in (-1, 0, 1) for dx in (0, 1)]     # d=5 col 0
RIGHT_TAPS = [(dy, dx) for dy in (-1, 0, 1) for dx in (-1, 0)]   # d=3 col 255
C6_TAPS = [(dy, dx) for dy in (0, 1) for dx in (-1, 0)]          # d=6 (0,255)
C2_TAPS = [(dy, dx) for dy in (-1, 0) for dx in (0, 1)]          # d=2 (255,0)
C8_TAPS = [(dy, dx) for dy in (0, 1) for dx in (0, 1)]           # d=8 (0,0)
C0_TAPS = [(dy, dx) for dy in (-1, 0) for dx in (-1, 0)]         # d=0 (255,255)

W_GROUPS = [
    (4, TAPS9), (7, TOP_TAPS), (1, BOT_TAPS), (5, LEFT_TAPS),
    (3, RIGHT_TAPS), (6, C6_TAPS), (2, C2_TAPS), (8, C8_TAPS), (0, C0_TAPS),
]
_offs = []
_acc = 0
for _d, _taps in W_GROUPS:
    _offs.append(_acc)
    _acc += len(_taps)
(MAIN_S, TOP_S, BOT_S, LEFT_S, RIGHT_S, C6_S, C2_S, C8_S, C0_S) = _offs
NW = _acc  # 49
NWM = 9    # main taps at the front of wt
# bias tile [128, NB]: column -> (value on partitions 0-63, on 64-127)
BIAS_PAIRS = [(4, 4), (5, 5), (3, 3), (7, 1), (8, 2), (6, 0)]
B_MAIN, B_LEFT, B_RIGHT, B_TOPBOT, B_C82, B_C60 = range(6)
NB = 6

EOUT = 516  # edges_out: [0:128] left, [128:256] right, [256:512] top|bottom,
            # [512:514] corner j01 (d8|d2), [514:516] corner j254/255 (d6|d0)

# input DMA chunk boundaries (slot index); first chunk small for fast start
XCHUNKS = [0, 10, 18, 46, 74, 102, SLOTS]
# output DMA every OG groups
OG = 2
# issue the edge computation after this many main-loop groups (image fully
# resident by then; keeps edge DMA latency off the kernel tail)
EDGE_AT = 18
# dummy matmul groups (structured exactly like main-loop groups) issued
# before the main loop to ramp the PE clock while the first image chunk is
# still in flight
WARMUP_GROUPS = 2

_CACHE = {}


def _interleaved_chains(nc, chains):
    """Issue accumulation chains with taps round-robined across chains so
    independent PE quadrants stream concurrently. Each chain is
    (psd, wslice, slot0, taps, rhs_fn)."""
    nmax = max(len(c[3]) for c in chains)
    for k in range(nmax):
        for (psd, wslice, slot0, taps, rhs_fn) in chains:
            if k >= len(taps):
                continue
            dy, dx = taps[k]
            nc.tensor.matmul(psd, wslice[:, ts(slot0 + k, 64)],
                             rhs_fn(dy, dx),
                             start=(k == 0), stop=(k == len(taps) - 1),
                             skip_group_check=True)


def _edges(nc, pp, constp, wtr, bias_t, Xv, edg_d, lo, up):
    """Border-pixel conv chains, reading the resident image tile."""
    esb = constp.tile([128, EOUT], F32)
    # side columns: 4 chains on quadrants (h0,c0) (h64,c64) (h0,c0) (h64,c64)
    pside = pp.tile([128, 256], F32, tag="ps1")
    pside2 = pp.tile([128, 256], F32, tag="ps2")
    _interleaved_chains(nc, [
        (pside[lo, 0:128], wtr[lo], LEFT_S, LEFT_TAPS,
         lambda dy, dx: Xv[lo, 1 + dy: 129 + dy, dx + 1]),
        (pside[up, 0:128], wtr[up], LEFT_S, LEFT_TAPS,
         lambda dy, dx: Xv[up, 1 + dy: 129 + dy, dx + 1]),
        (pside2[lo, 0:128], wtr[lo], RIGHT_S, RIGHT_TAPS,
         lambda dy, dx: Xv[lo, 1 + dy: 129 + dy, dx + 256]),
        (pside2[up, 0:128], wtr[up], RIGHT_S, RIGHT_TAPS,
         lambda dy, dx: Xv[up, 1 + dy: 129 + dy, dx + 256]),
    ])
    nc.scalar.activation(esb[:, 0:128], pside[:, 0:128], AF.Identity,
                         bias=bias_t[:, B_LEFT:B_LEFT + 1])
    nc.scalar.activation(esb[:, 128:256], pside2[:, 0:128], AF.Identity,
                         bias=bias_t[:, B_RIGHT:B_RIGHT + 1])
    # top row (d=7) on lo / bottom row (d=1) on up
    ptb = pp.tile([128, 256], F32, tag="ps1")
    _interleaved_chains(nc, [
        (ptb[lo, 0:256], wtr[lo], TOP_S, TOP_TAPS,
         lambda dy, dx: Xv[lo, 1 + dy, dx + 1: dx + 257]),
        (ptb[up, 0:256], wtr[up], BOT_S, BOT_TAPS,
         lambda dy, dx: Xv[up, 128 + dy, dx + 1: dx + 257]),
    ])
    nc.scalar.activation(esb[:, 256:512], ptb[:, 0:256], AF.Identity,
                         bias=bias_t[:, B_TOPBOT:B_TOPBOT + 1])
    # corners: (0,0) d8 / (255,0) d2 at cols 512:514;
    #          (0,255) d6 / (255,255) d0 at cols 514:516.
    # C8/C2 and C6/C0 share partitions, so interleaved accumulation epochs
    # must live in distinct PSUM banks (pcn vs pcn2).
    pcn = pp.tile([128, 256], F32, tag="ps2")
    pcn2 = pp.tile([128, 256], F32, tag="ps1")
    _interleaved_chains(nc, [
        (pcn[lo, 0:2], wtr[lo], C8_S, C8_TAPS,
         lambda dy, dx: Xv[lo, 1 + dy, dx + 1: dx + 3]),
        (pcn[up, 0:2], wtr[up], C2_S, C2_TAPS,
         lambda dy, dx: Xv[up, 128 + dy, dx + 1: dx + 3]),
        (pcn2[lo, 2:4], wtr[lo], C6_S, C6_TAPS,
         lambda dy, dx: Xv[lo, 1 + dy, 255 + dx: 257 + dx]),
        (pcn2[up, 2:4], wtr[up], C0_S, C0_TAPS,
         lambda dy, dx: Xv[up, 128 + dy, 255 + dx: 257 + dx]),
    ])
    nc.scalar.activation(esb[:, 512:514], pcn[:, 0:2], AF.Identity,
                         bias=bias_t[:, B_C82:B_C82 + 1])
    nc.scalar.activation(esb[:, 514:516], pcn2[:, 2:4], AF.Identity,
                         bias=bias_t[:, B_C60:B_C60 + 1])
    nc.sync.dma_start(edg_d[:], esb[:])


def _build():
    nc = bacc.Bacc("TRN2", target_bir_lowering=False, debug=False,
                   num_devices=NCORES)
    ip = nc.dram_tensor("img_prep", [128, XCOLS], BF16,
                        kind="ExternalInput").ap()
    wtm_d = nc.dram_tensor("wtm", [128, NWM * 64], BF16,
                           kind="ExternalInput").ap()
    wte_d = nc.dram_tensor("wte", [128, (NW - NWM) * 64], BF16,
                           kind="ExternalInput").ap()
    bias_d = nc.dram_tensor("bias", [128, NB], F32, kind="ExternalInput").ap()
    out_d = nc.dram_tensor("out", [128, 2, OCOLS], BF16,
                           kind="ExternalOutput").ap()
    edg_d = nc.dram_tensor("edges", [128, EOUT], F32,
                           kind="ExternalOutput").ap()

    lo, up = slice(0, 64), slice(64, 128)

    with tile.TileContext(nc) as tc:
        with (
            tc.tile_pool(name="const", bufs=1) as constp,
            tc.tile_pool(name="psmain", bufs=4, space="PSUM") as pp,
        ):
            # scratch for PE warmup (memset: the scheduler requires tiles to
            # be written before read; warmup then depends only on this)
            if WARMUP_GROUPS:
                scr = constp.tile([128, 512], BF16)
                nc.gpsimd.memset(scr[:], 0.0)

            wt = constp.tile([128, NW * 64], BF16)
            nc.sync.dma_start(wt[:, 0:NWM * 64], wtm_d[:])
            X = constp.tile([128, XCOLS], BF16)
            a0, b0 = XCHUNKS[0] * WPAD, XCHUNKS[1] * WPAD
            nc.sync.dma_start(X[:, a0:b0], ip[:, a0:b0])
            bias_t = constp.tile([128, NB], F32)
            nc.sync.dma_start(bias_t[:], bias_d[:])
            for ci in range(1, len(XCHUNKS) - 1):
                a, b = XCHUNKS[ci] * WPAD, XCHUNKS[ci + 1] * WPAD
                nc.sync.dma_start(X[:, a:b], ip[:, a:b])
            nc.sync.dma_start(wt[:, NWM * 64:], wte_d[:])
            wtr = wt[:]
            Xv = X[:].rearrange("p (t m) -> p t m", m=WPAD)

            # PE clock warmup on scratch data (no DMA dependencies);
            # structured exactly like a main-loop group so PSUM accumulation
            # epochs per bank+partition stay sequential.
            for _ in range(WARMUP_GROUPS):
                pw1 = pp.tile([128, 512], F32, tag="ps1")
                pw2 = pp.tile([128, 512], F32, tag="ps2")
                for k in range(9):
                    st, sp = (k == 0), (k == 8)
                    for (ph, po) in ((lo, slice(0, 64)), (up, slice(0, 64)),
                                     (lo, slice(64, 128)),
                                     (up, slice(64, 128))):
                        psd = (pw1 if ph == lo else pw2)
                        nc.tensor.matmul(psd[po, :], scr[ph, 0:64],
                                         scr[ph, :], start=st, stop=sp,
                                         skip_group_check=True)

            ot = constp.tile([128, 2 * OCOLS], BF16)
            otv = ot[:].rearrange("p (h c) -> p h c", h=2)

            # ---- dense interior conv (d=4) ----
            for g in range(NG):
                ps1 = pp.tile([128, 512], F32, tag="ps1")
                ps2 = pp.tile([128, 512], F32, tag="ps2")
                for k, (dy, dx) in enumerate(TAPS9):
                    st, sp = (k == 0), (k == 8)
                    for (ph, po, i) in ((lo, slice(0, 64), 4 * g),
                                        (up, slice(0, 64), 4 * g),
                                        (lo, slice(64, 128), 4 * g + 2),
                                        (up, slice(64, 128), 4 * g + 2)):
                        psd = (ps1 if ph == lo else ps2)
                        rhs = Xv[ph, i + 1 + dy: i + 3 + dy,
                                 dx + 1: dx + 257]
                        nc.tensor.matmul(
                            psd[po, :],
                            wtr[ph, ts(MAIN_S + k, 64)], rhs,
                            start=st, stop=sp, skip_group_check=True)
                # evacuate: bias add psum -> sbuf bf16; alternate engines
                blo = bias_t[:, B_MAIN:B_MAIN + 1]
                if g % 2 == 0:
                    nc.scalar.activation(ot[:, ts(g, 512)], ps1[:],
                                         AF.Identity, bias=blo)
                    nc.vector.tensor_scalar_add(ot[:, OCOLS + 512 * g:
                                                   OCOLS + 512 * (g + 1)],
                                                ps2[:], blo)
                else:
                    nc.vector.tensor_scalar_add(ot[:, ts(g, 512)],
                                                ps1[:], blo)
                    nc.scalar.activation(ot[:, OCOLS + 512 * g:
                                            OCOLS + 512 * (g + 1)], ps2[:],
                                         AF.Identity, bias=blo)
                if g >= NG - 2:
                    # final groups go out singly to shorten the kernel tail
                    a, b = g * 512, (g + 1) * 512
                    nc.sync.dma_start(out_d[:, :, a:b], otv[:, :, a:b])
                elif g % OG == OG - 1:
                    a, b = (g - OG + 1) * 512, (g + 1) * 512
                    nc.sync.dma_start(out_d[:, :, a:b], otv[:, :, a:b])
                if g == EDGE_AT - 1:
                    _edges(nc, pp, constp, wtr, bias_t, Xv, edg_d, lo, up)

    nc.compile()
    return nc


def _get_nc():
    if "nc" not in _CACHE:
        _CACHE["nc"] = _build()
    return _CACHE["nc"]


def _prep_img(imgc):
    """[64,256,256] f32 -> [128, XCOLS] whole-image padded bf16 layout."""
    ipv = np.zeros((2, 64, SLOTS, WPAD), BF)
    ipv[0, :, 1:130, 1:257] = imgc[:, 0:129, :]     # lo: rows -1..128
    ipv[1, :, 0:129, 1:257] = imgc[:, 127:256, :]   # up: rows 127..256
    return np.ascontiguousarray(ipv.reshape(128, XCOLS))


def _prep_wt(weights):
    wt = np.zeros((128, NW, 64), BF)
    for (d, taps), base in zip(W_GROUPS, _offs):
        for k, (dy, dx) in enumerate(taps):
            m = weights[d][:, :, dy + 1, dx + 1].T  # [cin, cout]
            wt[0:64, base + k] = m
            wt[64:128, base + k] = m
    return np.ascontiguousarray(wt.reshape(128, NW * 64))


def _prep_bias(bias):
    bs = np.zeros((128, NB), np.float32)
    for c, (dl, du) in enumerate(BIAS_PAIRS):
        bs[0:64, c] = bias[dl]
        bs[64:128, c] = bias[du]
    return bs


def _make_in_maps(img, weights, bias):
    img = np.asarray(img, np.float32)
    wt = _prep_wt(np.asarray(weights, np.float32))
    wtm = np.ascontiguousarray(wt[:, :NWM * 64])
    wte = np.ascontiguousarray(wt[:, NWM * 64:])
    bs = _prep_bias(np.asarray(bias, np.float32))
    return [{"img_prep": _prep_img(img[c]), "wtm": wtm, "wte": wte,
             "bias": bs}
            for c in range(NCORES)]


def _unprep_out(o, e):
    """Assemble [C,H,W] f32 from dense bf16 out + f32 edge overlay."""
    v = o.reshape(2, 64, 2, NG, 2, 256)  # pg c h g r w
    out = np.ascontiguousarray(
        v.transpose(1, 2, 3, 0, 4, 5).reshape(C, H, W)).astype(np.float32)
    Lv = np.concatenate([e[0:64, 0:128], e[64:128, 0:128]], axis=1)
    Rv = np.concatenate([e[0:64, 128:256], e[64:128, 128:256]], axis=1)
    out[:, 1:255, 0] = Lv[:, 1:255]
    out[:, 1:255, 255] = Rv[:, 1:255]
    out[:, 0, 1:255] = e[0:64, 257:511]
    out[:, 255, 1:255] = e[64:128, 257:511]
    out[:, 0, 0] = e[0:64, 512]
    out[:, 255, 0] = e[64:128, 512]
    out[:, 0, 255] = e[0:64, 515]
    out[:, 255, 255] = e[64:128, 515]
    return out


def kernel(img, weights, bias):
    nc = _get_nc()
    in_maps = _make_in_maps(img, weights, bias)
    res = run_bass_kernel_spmd(nc, in_maps, list(range(NCORES)))
    return np.stack([_unprep_out(res.results[c]["out"],
                                 res.results[c]["edges"])
                     for c in range(NCORES)])
